# revision 11
# baseline (speedup 1.0000x reference)
"""BitLinear158 Trainium2 kernel.

Reference computation:
    gamma = mean(|W|)
    Wq    = clip(round(W / (gamma + 1e-5)), -1, 1)      # ternary {-1, 0, +1}
    out   = x @ Wq.T + b                                # x: [8, 4096, 2048]

Sharding: data-parallel over the batch dim (8 batches -> 8 cores). Each core
gets x[i] (host-transposed to k-major so the contraction dim lands on SBUF
partitions with unit-stride DMA), the full W (host-transposed, same reason)
and b, and computes its 4096-token slice of the output. gamma is computed
redundantly per-core from the full W -- no collectives needed.

Device pipeline per core:
  pass 1: stream WT (16 MiB), fused |.| + row-reduce -> partition_all_reduce
          -> gamma -> s = 1/(gamma+eps) (reciprocal + 1 Newton step)
  pass 2: re-stream WT, ternarize via (z>0.5) - (z<-0.5) into resident
          WqT bf16 tiles (exact in bf16)
  main:   for each 128-token tile: DMA xT slice, cast bf16 (ACT),
          16x4 matmuls accumulating into PSUM [128, 2048] (bf16 inputs,
          fp32 accumulate), bias-add on PSUM->SBUF evacuation (DVE),
          DMA out.
"""

from contextlib import ExitStack

import numpy as np

import concourse.bacc as bacc
import concourse.bass as bass
import concourse.mybir as mybir
import concourse.tile as tile
from concourse import library_config
from concourse.bass_isa import ReduceOp
from concourse.bass_utils import run_bass_kernel_spmd

P = 128
B, S, D_IN, D_OUT = 8, 4096, 2048, 2048
N_CORES = 8
TOK = (B * S) // N_CORES          # 4096 tokens per core
KT = D_IN // P                    # 16 k-tiles
TT = TOK // P                     # 32 token tiles
NC_CHUNK = 512                    # matmul moving free dim (1 PSUM bank fp32)
OC = D_OUT // NC_CHUNK            # 4 output chunks
W_ELEMS = D_OUT * D_IN            # 2**22 (power of 2: S/N == S*(1/N) exactly)
EPS = 1e-5

F32 = mybir.dt.float32
BF16 = mybir.dt.bfloat16
MULT = mybir.AluOpType.mult
ADD = mybir.AluOpType.add
IS_GT = mybir.AluOpType.is_gt
AX_X = mybir.AxisListType.X


def build_nc() -> bass.Bass:
    nc = bacc.Bacc(None, target_bir_lowering=False)
    xT = nc.dram_tensor("xT", [D_IN, TOK], F32, kind="ExternalInput")
    WT = nc.dram_tensor("WT", [D_IN, D_OUT], F32, kind="ExternalInput")
    b = nc.dram_tensor("b", [D_OUT], F32, kind="ExternalInput")
    out = nc.dram_tensor("out", [TOK, D_OUT], F32, kind="ExternalOutput")

    with tile.TileContext(nc) as tc, ExitStack() as ctx:
        wpool = ctx.enter_context(tc.tile_pool(name="wpass", bufs=3))
        spool = ctx.enter_context(tc.tile_pool(name="scalars", bufs=1))
        qpool = ctx.enter_context(tc.tile_pool(name="qtmp", bufs=4))
        wqpool = ctx.enter_context(tc.tile_pool(name="wq", bufs=KT))
        xpool = ctx.enter_context(tc.tile_pool(name="x", bufs=3))
        xbpool = ctx.enter_context(tc.tile_pool(name="xb", bufs=3))
        opool = ctx.enter_context(tc.tile_pool(name="osb", bufs=3))
        pspool = ctx.enter_context(
            tc.tile_pool(name="psum", bufs=2, space="PSUM")
        )

        # Bias replicated to all partitions (partition-broadcast DMA).
        bias_sb = spool.tile([P, D_OUT], F32)
        b_row = b[:].rearrange("(o d) -> o d", o=1)
        nc.gpsimd.dma_start(bias_sb[:], b_row.to_broadcast((P, D_OUT)))

        # ---- pass 1: gamma = mean |W| ----
        partials = spool.tile([P, KT], F32)
        for kt in range(KT):
            wt = wpool.tile([P, D_OUT], F32, tag="wt")
            nc.gpsimd.dma_start(wt[:], WT[kt * P : (kt + 1) * P, :])
            nc.vector.reduce_sum(
                partials[:, kt : kt + 1],
                wt[:],
                axis=AX_X,
                apply_absolute_value=True,
            )
        colsum = spool.tile([P, 1], F32)
        nc.vector.reduce_sum(colsum[:], partials[:], axis=AX_X)
        # Partition reduce + broadcast in one PE op: ones.T @ colsum puts
        # sum(colsum) on every partition.
        ones_sq = spool.tile([P, P], F32)
        nc.vector.memset(ones_sq[:], 1.0)
        total_ps = pspool.tile([P, D_OUT], F32, tag="ps")
        nc.tensor.matmul(
            total_ps[:, 0:1], ones_sq[:], colsum[:], start=True, stop=True
        )
        total = spool.tile([P, 1], F32)
        nc.vector.tensor_copy(total[:], total_ps[:, 0:1])

        # s = 1/(gamma + eps); one Newton step tightens vector.reciprocal
        # to ~correctly-rounded so W*s tracks the reference's W/(gamma+eps).
        geps = spool.tile([P, 1], F32)
        nc.vector.tensor_scalar(geps[:], total[:], 1.0 / W_ELEMS, EPS, MULT, ADD)
        r0 = spool.tile([P, 1], F32)
        nc.vector.reciprocal(r0[:], geps[:])
        t1 = spool.tile([P, 1], F32)
        nc.vector.tensor_tensor(t1[:], geps[:], r0[:], op=MULT)
        t2 = spool.tile([P, 1], F32)
        nc.vector.tensor_scalar(t2[:], t1[:], -1.0, 2.0, MULT, ADD)
        s_pos = spool.tile([P, 1], F32)
        nc.vector.tensor_tensor(s_pos[:], r0[:], t2[:], op=MULT)
        s_neg = spool.tile([P, 1], F32)
        nc.vector.tensor_scalar_mul(s_neg[:], s_pos[:], -1.0)

        # ---- pass 2: WqT = (z > 0.5) - (z < -0.5), z = W*s  (bf16, exact) ----
        wq_tiles = []
        for kt in range(KT):
            wt = wpool.tile([P, D_OUT], F32, tag="wt")
            nc.gpsimd.dma_start(wt[:], WT[kt * P : (kt + 1) * P, :])
            ga = qpool.tile([P, D_OUT], BF16, tag="q")
            gb = qpool.tile([P, D_OUT], BF16, tag="q")
            nc.vector.tensor_scalar(ga[:], wt[:], s_pos[:], 0.5, MULT, IS_GT)
            nc.vector.tensor_scalar(gb[:], wt[:], s_neg[:], 0.5, MULT, IS_GT)
            wq = wqpool.tile([P, D_OUT], BF16, tag="wq")
            nc.vector.tensor_sub(wq[:], ga[:], gb[:])
            wq_tiles.append(wq)

        # ---- main: out[t, :] = x[t, :] @ WqT + b ----
        xT_v = xT.rearrange("(a p) t -> p a t", p=P)  # [128, KT, TOK]
        for tt in range(TT):
            xt = xpool.tile([P, KT, P], F32, tag="xt")
            nc.gpsimd.dma_start(xt[:], xT_v[:, :, tt * P : (tt + 1) * P])
            xb = xbpool.tile([P, KT, P], BF16, tag="xb")
            nc.scalar.copy(xb[:], xt[:])

            ps = pspool.tile([P, D_OUT], F32, tag="ps")
            for kt in range(KT):
                for oc in range(OC):
                    nc.tensor.matmul(
                        ps[:, oc * NC_CHUNK : (oc + 1) * NC_CHUNK],
                        xb[:, kt, :],
                        wq_tiles[kt][:, oc * NC_CHUNK : (oc + 1) * NC_CHUNK],
                        start=(kt == 0),
                        stop=(kt == KT - 1),
                    )

            osb = opool.tile([P, D_OUT], F32, tag="osb")
            nc.vector.tensor_add(osb[:], ps[:], bias_sb[:])
            nc.gpsimd.dma_start(out[tt * P : (tt + 1) * P, :], osb[:])

    nc.finalize()
    return nc


_NC_CACHE: list = []


def _get_nc() -> bass.Bass:
    if not _NC_CACHE:
        _NC_CACHE.append(build_nc())
    return _NC_CACHE[0]


def make_in_maps(x: np.ndarray, W: np.ndarray, b: np.ndarray):
    x = np.asarray(x, dtype=np.float32).reshape(N_CORES, TOK, D_IN)
    W = np.asarray(W, dtype=np.float32)
    b = np.asarray(b, dtype=np.float32)
    WT = np.ascontiguousarray(W.T)
    return [
        {"xT": np.ascontiguousarray(x[c].T), "WT": WT, "b": b}
        for c in range(N_CORES)
    ]


def run(x, W, b, **spmd_kwargs):
    """Run the SPMD kernel; returns (full_output, BassKernelResults)."""
    nc = _get_nc()
    in_maps = make_in_maps(x, W, b)
    res = run_bass_kernel_spmd(nc, in_maps, list(range(N_CORES)), **spmd_kwargs)
    out = np.stack([res.results[c]["out"] for c in range(N_CORES)], axis=0)
    return out.reshape(B, S, D_OUT), res


def kernel(x, W, b):
    out, _ = run(x, W, b)
    return out


# revision 13
# speedup vs baseline: 1.0024x; 1.0024x over previous
"""BitLinear158 Trainium2 kernel.

Reference computation:
    gamma = mean(|W|)
    Wq    = clip(round(W / (gamma + 1e-5)), -1, 1)      # ternary {-1, 0, +1}
    out   = x @ Wq.T + b                                # x: [8, 4096, 2048]

Sharding: data-parallel over the batch dim (8 batches -> 8 cores). Each core
gets x[i] (host-transposed to k-major so the contraction dim lands on SBUF
partitions with unit-stride DMA), the full W (host-transposed, same reason)
and b, and computes its 4096-token slice of the output. gamma is computed
redundantly per-core from the full W -- no collectives needed.

Device pipeline per core:
  pass 1: stream WT (16 MiB), fused |.| + row-reduce -> partition_all_reduce
          -> gamma -> s = 1/(gamma+eps) (reciprocal + 1 Newton step)
  pass 2: re-stream WT, ternarize via (z>0.5) - (z<-0.5) into resident
          WqT bf16 tiles (exact in bf16)
  main:   for each 128-token tile: DMA xT slice, cast bf16 (ACT),
          16x4 matmuls accumulating into PSUM [128, 2048] (bf16 inputs,
          fp32 accumulate), bias-add on PSUM->SBUF evacuation (DVE),
          DMA out.
"""

from contextlib import ExitStack

import numpy as np

import concourse.bacc as bacc
import concourse.bass as bass
import concourse.mybir as mybir
import concourse.tile as tile
from concourse import library_config
from concourse.bass_isa import ReduceOp
from concourse.bass_utils import run_bass_kernel_spmd

P = 128
B, S, D_IN, D_OUT = 8, 4096, 2048, 2048
N_CORES = 8
TOK = (B * S) // N_CORES          # 4096 tokens per core
KT = D_IN // P                    # 16 k-tiles
TT = TOK // P                     # 32 token tiles
NC_CHUNK = 512                    # matmul moving free dim (1 PSUM bank fp32)
OC = D_OUT // NC_CHUNK            # 4 output chunks
W_ELEMS = D_OUT * D_IN            # 2**22 (power of 2: S/N == S*(1/N) exactly)
EPS = 1e-5

F32 = mybir.dt.float32
BF16 = mybir.dt.bfloat16
MULT = mybir.AluOpType.mult
ADD = mybir.AluOpType.add
IS_GT = mybir.AluOpType.is_gt
AX_X = mybir.AxisListType.X


def build_nc() -> bass.Bass:
    nc = bacc.Bacc(None, target_bir_lowering=False)
    xT = nc.dram_tensor("xT", [D_IN, TOK], F32, kind="ExternalInput")
    WT = nc.dram_tensor("WT", [D_IN, D_OUT], F32, kind="ExternalInput")
    b = nc.dram_tensor("b", [D_OUT], F32, kind="ExternalInput")
    out = nc.dram_tensor("out", [TOK, D_OUT], F32, kind="ExternalOutput")

    with tile.TileContext(nc) as tc, ExitStack() as ctx:
        wpool = ctx.enter_context(tc.tile_pool(name="wpass", bufs=3))
        spool = ctx.enter_context(tc.tile_pool(name="scalars", bufs=1))
        qpool = ctx.enter_context(tc.tile_pool(name="qtmp", bufs=4))
        wqpool = ctx.enter_context(tc.tile_pool(name="wq", bufs=KT))
        xbpool = ctx.enter_context(tc.tile_pool(name="xb", bufs=4))
        opool = ctx.enter_context(tc.tile_pool(name="osb", bufs=3))
        pspool = ctx.enter_context(
            tc.tile_pool(name="psum", bufs=2, space="PSUM")
        )

        # Bias replicated to all partitions (partition-broadcast DMA).
        bias_sb = spool.tile([P, D_OUT], F32)
        b_row = b[:].rearrange("(o d) -> o d", o=1)
        nc.gpsimd.dma_start(bias_sb[:], b_row.to_broadcast((P, D_OUT)))

        # ---- pass 1: gamma = mean |W| ----
        partials = spool.tile([P, KT], F32)
        for kt in range(KT):
            wt = wpool.tile([P, D_OUT], F32, tag="wt")
            nc.gpsimd.dma_start(wt[:], WT[kt * P : (kt + 1) * P, :])
            nc.vector.reduce_sum(
                partials[:, kt : kt + 1],
                wt[:],
                axis=AX_X,
                apply_absolute_value=True,
            )
        colsum = spool.tile([P, 1], F32)
        nc.vector.reduce_sum(colsum[:], partials[:], axis=AX_X)
        # Partition reduce + broadcast in one PE op: ones.T @ colsum puts
        # sum(colsum) on every partition.
        ones_sq = spool.tile([P, P], F32)
        nc.vector.memset(ones_sq[:], 1.0)
        total_ps = pspool.tile([P, D_OUT], F32, tag="ps")
        nc.tensor.matmul(
            total_ps[:, 0:1], ones_sq[:], colsum[:], start=True, stop=True
        )
        total = spool.tile([P, 1], F32)
        nc.vector.tensor_copy(total[:], total_ps[:, 0:1])

        # s = 1/(gamma + eps); one Newton step tightens vector.reciprocal
        # to ~correctly-rounded so W*s tracks the reference's W/(gamma+eps).
        geps = spool.tile([P, 1], F32)
        nc.vector.tensor_scalar(geps[:], total[:], 1.0 / W_ELEMS, EPS, MULT, ADD)
        r0 = spool.tile([P, 1], F32)
        nc.vector.reciprocal(r0[:], geps[:])
        t1 = spool.tile([P, 1], F32)
        nc.vector.tensor_tensor(t1[:], geps[:], r0[:], op=MULT)
        t2 = spool.tile([P, 1], F32)
        nc.vector.tensor_scalar(t2[:], t1[:], -1.0, 2.0, MULT, ADD)
        s_pos = spool.tile([P, 1], F32)
        nc.vector.tensor_tensor(s_pos[:], r0[:], t2[:], op=MULT)
        s_neg = spool.tile([P, 1], F32)
        nc.vector.tensor_scalar_mul(s_neg[:], s_pos[:], -1.0)

        # ---- pass 2: WqT = (z > 0.5) - (z < -0.5), z = W*s  (bf16, exact) ----
        wq_tiles = []
        for kt in range(KT):
            wt = wpool.tile([P, D_OUT], F32, tag="wt")
            nc.gpsimd.dma_start(wt[:], WT[kt * P : (kt + 1) * P, :])
            ga = qpool.tile([P, D_OUT], BF16, tag="q")
            gb = qpool.tile([P, D_OUT], BF16, tag="q")
            nc.vector.tensor_scalar(ga[:], wt[:], s_pos[:], 0.5, MULT, IS_GT)
            nc.vector.tensor_scalar(gb[:], wt[:], s_neg[:], 0.5, MULT, IS_GT)
            wq = wqpool.tile([P, D_OUT], BF16, tag="wq")
            nc.vector.tensor_sub(wq[:], ga[:], gb[:])
            wq_tiles.append(wq)

        # ---- main: out[t, :] = x[t, :] @ WqT + b ----
        xT_v = xT.rearrange("(a p) t -> p a t", p=P)  # [128, KT, TOK]
        for tt in range(TT):
            # SWDGE DMA casts fp32 -> bf16 inline (RNE), so the activations
            # land in SBUF already in matmul dtype with no compute-engine work.
            xb = xbpool.tile([P, KT, P], BF16, tag="xb")
            nc.gpsimd.dma_start(xb[:], xT_v[:, :, tt * P : (tt + 1) * P])

            ps = pspool.tile([P, D_OUT], F32, tag="ps")
            for kt in range(KT):
                for oc in range(OC):
                    nc.tensor.matmul(
                        ps[:, oc * NC_CHUNK : (oc + 1) * NC_CHUNK],
                        xb[:, kt, :],
                        wq_tiles[kt][:, oc * NC_CHUNK : (oc + 1) * NC_CHUNK],
                        start=(kt == 0),
                        stop=(kt == KT - 1),
                    )

            osb = opool.tile([P, D_OUT], F32, tag="osb")
            nc.vector.tensor_add(osb[:], ps[:], bias_sb[:])
            nc.gpsimd.dma_start(out[tt * P : (tt + 1) * P, :], osb[:])

    nc.finalize()
    return nc


_NC_CACHE: list = []


def _get_nc() -> bass.Bass:
    if not _NC_CACHE:
        _NC_CACHE.append(build_nc())
    return _NC_CACHE[0]


def make_in_maps(x: np.ndarray, W: np.ndarray, b: np.ndarray):
    x = np.asarray(x, dtype=np.float32).reshape(N_CORES, TOK, D_IN)
    W = np.asarray(W, dtype=np.float32)
    b = np.asarray(b, dtype=np.float32)
    WT = np.ascontiguousarray(W.T)
    return [
        {"xT": np.ascontiguousarray(x[c].T), "WT": WT, "b": b}
        for c in range(N_CORES)
    ]


def run(x, W, b, **spmd_kwargs):
    """Run the SPMD kernel; returns (full_output, BassKernelResults)."""
    nc = _get_nc()
    in_maps = make_in_maps(x, W, b)
    res = run_bass_kernel_spmd(nc, in_maps, list(range(N_CORES)), **spmd_kwargs)
    out = np.stack([res.results[c]["out"] for c in range(N_CORES)], axis=0)
    return out.reshape(B, S, D_OUT), res


def kernel(x, W, b):
    out, _ = run(x, W, b)
    return out
